# revision 22
# baseline (speedup 1.0000x reference)
# Trainium2 Bass kernel for nn_Connection_geognn_17076789969601.
#
# Math (per sample row of input_ [N, 128], x = row[:64], v = row[64:]):
#   h  = tanh(W1 @ x + b1)                  # [128]
#   Wm = tanh(W2 @ h + b2).reshape(64, 4)   # [64, 4]
#   u  = v @ Wm ;  H = sum(u^2)
#   output = [dH/dx, -dH/dv]
#
# Backward (per sample), with T = tanh(A2) in W2-row-permuted order so that
# column j of Wm occupies rows [64j, 64j+64):
#   dv_out = -2 Wm u
#   dA2    = 2 (v x u) * (1 - T^2) = Q - M,  Q = 2u*v,  M = 2u*v*T^2
#   dh     = W2r^T dA2 ;  dA1 = (1-h^2)*dh ;  dx = W1^T dA1
#
# v4 design (evolved from trace analysis):
#   - v1->v2: no GpSimd elementwise (shared SBUF port poisons DVE), wide
#     [128, 4096] DVE products over groups of 4 subtiles, host-side v
#     replication, fewer DMAs/semaphores.  DVE 300->224us, Act 234->211us.
#   - v4: S,M wides before Q and dh accumulates M-terms first so backward
#     matmuls start ~2.3us earlier; dx/dv matmuls column-tiled into disjoint
#     PE col-strips (concurrent, saves ~1 pass per subtile); dh1 and outq
#     share one 2-slot PSUM tag (frees a bank pair); a1 matmuls for adjacent
#     subtiles row-tiled (K=64) into strips (0,0)/(64,0) and run
#     concurrently on the PE using a host-packed [128, n/2] x layout.
#   - Engine split: Act {tanh x3, Rc pair-copy, 1/2 out-copies};
#     DVE {P,S,M,Q wides, h1sq, dA1 STT, 1/2 out-copies}; PE {matmuls}.
#
# Sharding: pure data parallel over 8 NeuronCores, batch 262144 -> 8 x 32768,
# weights replicated.

import sys

sys.path.insert(0, "/opt/trn_rl_repo")

import numpy as np
import ml_dtypes

import concourse.bass as bass
import concourse.bacc as bacc
import concourse.tile as tile
import concourse.mybir as mybir
from concourse.bass_utils import run_bass_kernel_spmd

F32 = mybir.dt.float32
BF16 = mybir.dt.bfloat16
AF = mybir.ActivationFunctionType
ALU = mybir.AluOpType

D = 64
RANK = 4
N_TOTAL = 262144
N_CORES = 8
N_ROWS = N_TOTAL // N_CORES  # 32768 per core
B = 512                      # samples per subtile (PSUM-bank sized)
G = 2                        # subtiles per wide group


def build_program(n_rows=N_ROWS, b=B, g=G):
    ng = n_rows // (b * g)   # wide groups
    gb = g * b               # samples per group (2048)
    nc = bacc.Bacc()

    # x packed pairwise: rows 0:64 = x^T of even subtiles, 64:128 odd.
    xtp = nc.declare_dram_parameter("xt", [128, n_rows // 2], BF16,
                                    isOutput=False)
    vrp = nc.declare_dram_parameter("vr", [128, n_rows], BF16, isOutput=False)
    w1t2 = nc.declare_dram_parameter("w1t2", [128, 128], BF16, isOutput=False)
    w2ta = nc.declare_dram_parameter("w2ta", [128, 128], BF16, isOutput=False)
    w2tb = nc.declare_dram_parameter("w2tb", [128, 128], BF16, isOutput=False)
    w2pa = nc.declare_dram_parameter("w2pa", [128, 128], BF16, isOutput=False)
    w2pb = nc.declare_dram_parameter("w2pb", [128, 128], BF16, isOutput=False)
    w2ma = nc.declare_dram_parameter("w2ma", [128, 128], BF16, isOutput=False)
    w2mb = nc.declare_dram_parameter("w2mb", [128, 128], BF16, isOutput=False)
    w1n = nc.declare_dram_parameter("w1n", [128, 64], BF16, isOutput=False)
    mblk = nc.declare_dram_parameter("mblk", [128, 128], BF16, isOutput=False)
    msum = nc.declare_dram_parameter("msum", [128, 64], BF16, isOutput=False)
    b1p = nc.declare_dram_parameter("b1", [128, 1], F32, isOutput=False)
    b2ap = nc.declare_dram_parameter("b2a", [128, 1], F32, isOutput=False)
    b2bp = nc.declare_dram_parameter("b2b", [128, 1], F32, isOutput=False)
    outp = nc.declare_dram_parameter("out", [128, n_rows], BF16, isOutput=True)

    with tile.TileContext(nc) as tc:
        with (
            tc.tile_pool(name="const", bufs=1) as cp,
            tc.tile_pool(name="sb", bufs=3) as sb,
            tc.tile_pool(name="ps", bufs=1, space="PSUM") as ps,
        ):
            c_w1t2 = cp.tile([128, 128], BF16, tag="w1t2")
            c_w2ta = cp.tile([128, 128], BF16, tag="w2ta")
            c_w2tb = cp.tile([128, 128], BF16, tag="w2tb")
            c_w2pa = cp.tile([128, 128], BF16, tag="w2pa")
            c_w2pb = cp.tile([128, 128], BF16, tag="w2pb")
            c_w2ma = cp.tile([128, 128], BF16, tag="w2ma")
            c_w2mb = cp.tile([128, 128], BF16, tag="w2mb")
            c_w1n = cp.tile([128, 64], BF16, tag="w1n")
            c_mblk = cp.tile([128, 128], BF16, tag="mblk")
            c_msum = cp.tile([128, 64], BF16, tag="msum")
            c_b1 = cp.tile([128, 1], F32, tag="b1")
            c_b2a = cp.tile([128, 1], F32, tag="b2a")
            c_b2b = cp.tile([128, 1], F32, tag="b2b")
            for t_, p_ in (
                (c_w1t2, w1t2), (c_w2ta, w2ta), (c_w2tb, w2tb),
                (c_w2pa, w2pa), (c_w2pb, w2pb), (c_w2ma, w2ma),
                (c_w2mb, w2mb), (c_w1n, w1n), (c_mblk, mblk),
                (c_msum, msum), (c_b1, b1p), (c_b2a, b2ap), (c_b2b, b2bp),
            ):
                nc.sync.dma_start(t_[:], p_[:])

            for gi in range(ng):
                # ---- group input loads (contiguous HBM->SBUF) ----
                xt = sb.tile([128, gb // 2], BF16, tag="XT")
                vr = sb.tile([128, gb], BF16, tag="VR")
                nc.sync.dma_start(xt[:], xtp[:, bass.ts(gi, gb // 2)])
                nc.sync.dma_start(vr[:], vrp[:, bass.ts(gi, gb)])

                h = sb.tile([128, gb], BF16, tag="H")
                t2 = sb.tile([128, 2 * gb], BF16, tag="T2")

                # ---- forward: paired a1 (row-tiled K=64), tanh, a2, T2 ----
                for p in range(g // 2):
                    psl = bass.ts(p, b)          # pair cols in packed x
                    a1e = ps.tile([128, b], F32, tag="a1", bufs=2)
                    a1o = ps.tile([128, b], F32, tag="a1", bufs=2)
                    nc.tensor.matmul(a1e[:], c_w1t2[0:64, :], xt[0:64, psl],
                                     start=True, stop=True,
                                     tile_position=(0, 0))
                    nc.tensor.matmul(a1o[:], c_w1t2[64:128, :], xt[64:128, psl],
                                     start=True, stop=True,
                                     tile_position=(64, 0))
                    for k, a1 in ((0, a1e), (1, a1o)):
                        s = 2 * p + k
                        sl = bass.ts(s, b)
                        nc.scalar.activation(h[:, sl], a1[:], AF.Tanh,
                                             bias=c_b1[:, 0:1])
                        a2 = ps.tile([128, 2 * b], F32, tag="a2")
                        nc.tensor.matmul(a2[:, 0:b], c_w2ta[:], h[:, sl],
                                         start=True, stop=True)
                        nc.tensor.matmul(a2[:, b:2 * b], c_w2tb[:], h[:, sl],
                                         start=True, stop=True)
                        nc.scalar.activation(t2[:, 2 * s * b:(2 * s + 1) * b],
                                             a2[:, 0:b], AF.Tanh,
                                             bias=c_b2a[:, 0:1])
                        nc.scalar.activation(
                            t2[:, (2 * s + 1) * b:(2 * s + 2) * b],
                            a2[:, b:2 * b], AF.Tanh, bias=c_b2b[:, 0:1])

                # ---- P = T2 * vrep, split per subtile so R/Rc of subtile 0
                # can start before subtile 1's T2 is done ----
                pt = sb.tile([128, 2 * gb], BF16, tag="P")
                t2v = t2[:].rearrange("p (s h c) -> p s h c", s=g, h=2)
                pv = pt[:].rearrange("p (s h c) -> p s h c", s=g, h=2)
                vex = vr[:].rearrange("p (s c) -> p s c", s=g) \
                    .unsqueeze(2).broadcast_to((128, g, 2, b))
                for s in range(g):
                    nc.vector.tensor_mul(pv[:, s], t2v[:, s], vex[:, s])

                # ---- R = mblk @ P (2-bank PSUM pair), Rc copy (1 Act op) --
                rc = sb.tile([128, 2 * gb], BF16, tag="RC")
                for s in range(g):
                    r2 = ps.tile([128, 2 * b], F32, tag="r2")
                    nc.tensor.matmul(r2[:, 0:b], c_mblk[:],
                                     pt[:, 2 * s * b:(2 * s + 1) * b],
                                     start=True, stop=True)
                    nc.tensor.matmul(r2[:, b:2 * b], c_mblk[:],
                                     pt[:, (2 * s + 1) * b:(2 * s + 2) * b],
                                     start=True, stop=True)
                    nc.scalar.copy(rc[:, 2 * s * b:(2 * s + 2) * b], r2[:])

                # ---- wide S, M first (dh starts sooner), then Q ----
                st = sb.tile([128, 2 * gb], BF16, tag="S")
                nc.vector.tensor_mul(st[:], rc[:], t2[:])
                mt = sb.tile([128, 2 * gb], BF16, tag="M")
                nc.vector.tensor_mul(mt[:], st[:], pt[:])
                qt = sb.tile([128, 2 * gb], BF16, tag="Q")
                qv = qt[:].rearrange("p (s h c) -> p s h c", s=g, h=2)
                rcv = rc[:].rearrange("p (s h c) -> p s h c", s=g, h=2)
                nc.vector.tensor_mul(qv, rcv, vex)

                # ---- wide h1sq = h * h (2/3 DVE, 1/3 Act for balance) ----
                h1sq = sb.tile([128, gb], BF16, tag="H1SQ")
                if gi % 3 == 0:
                    nc.scalar.activation(h1sq[:], h[:], AF.Square)
                else:
                    nc.vector.tensor_mul(h1sq[:], h[:], h[:])

                # ---- backward per subtile; dh1/outq share one PSUM tag ----
                da1 = sb.tile([128, gb], BF16, tag="DA1")
                outs = sb.tile([128, gb], BF16, tag="OUTS")
                for s in range(g):
                    sl = bass.ts(s, b)
                    sa = slice(2 * s * b, (2 * s + 1) * b)
                    sbb = slice((2 * s + 1) * b, (2 * s + 2) * b)
                    dh1 = ps.tile([128, b], F32, tag="work", bufs=2)
                    nc.tensor.matmul(dh1[:], c_w2ma[:], mt[:, sa],
                                     start=True, stop=False)
                    nc.tensor.matmul(dh1[:], c_w2mb[:], mt[:, sbb],
                                     start=False, stop=False)
                    nc.tensor.matmul(dh1[:], c_w2pa[:], qt[:, sa],
                                     start=False, stop=False)
                    nc.tensor.matmul(dh1[:], c_w2pb[:], qt[:, sbb],
                                     start=False, stop=True)
                    nc.vector.scalar_tensor_tensor(
                        da1[:, sl], h1sq[:, sl], 1.0, dh1[:],
                        ALU.subtract, ALU.mult)
                    outq = ps.tile([128, b], F32, tag="work", bufs=2)
                    nc.tensor.matmul(outq[0:64, :], c_w1n[:], da1[:, sl],
                                     start=True, stop=True,
                                     tile_position=(0, 0))
                    nc.tensor.matmul(outq[64:128, :], c_msum[:], st[:, sa],
                                     start=True, stop=False,
                                     tile_position=(0, 64))
                    nc.tensor.matmul(outq[64:128, :], c_msum[:], st[:, sbb],
                                     start=False, stop=True,
                                     tile_position=(0, 64))
                    if s % 2 == 0:
                        nc.scalar.copy(outs[:, sl], outq[:])
                    else:
                        nc.vector.tensor_copy(outs[:, sl], outq[:])

                nc.sync.dma_start(outp[:, bass.ts(gi, gb)], outs[:])

    nc.finalize()
    return nc


def make_consts(W1, b1, W2, b2):
    """Host-side constant preparation (permutes W2 rows, folds signs)."""
    bf = ml_dtypes.bfloat16
    W1 = np.asarray(W1, np.float32)
    b1 = np.asarray(b1, np.float32)
    W2 = np.asarray(W2, np.float32)
    b2 = np.asarray(b2, np.float32)
    perm = np.empty(RANK * D, np.int64)
    for j in range(RANK):
        for i in range(D):
            perm[j * D + i] = i * RANK + j
    W2r = W2[perm, :]
    b2r = b2[perm]
    mblk = np.zeros((128, 128), np.float32)
    mblk[:64, :64] = 2.0
    mblk[64:, 64:] = 2.0
    msum = np.zeros((128, 64), np.float32)
    for i in range(64):
        msum[i, i] = -1.0
        msum[64 + i, i] = -1.0
    w1t = np.ascontiguousarray(W1.T)                 # [64, 128]
    w1t2 = np.concatenate([w1t, w1t], axis=0)        # [128, 128]
    return {
        "w1t2": w1t2.astype(bf),
        "w2ta": np.ascontiguousarray(W2r[:128].T).astype(bf),
        "w2tb": np.ascontiguousarray(W2r[128:].T).astype(bf),
        "w2pa": np.ascontiguousarray(W2r[:128]).astype(bf),
        "w2pb": np.ascontiguousarray(W2r[128:]).astype(bf),
        "w2ma": np.ascontiguousarray(-W2r[:128]).astype(bf),
        "w2mb": np.ascontiguousarray(-W2r[128:]).astype(bf),
        "w1n": np.ascontiguousarray(-W1).astype(bf),
        "mblk": mblk.astype(bf),
        "msum": msum.astype(bf),
        "b1": b1.reshape(128, 1).astype(np.float32),
        "b2a": b2r[:128].reshape(128, 1).astype(np.float32),
        "b2b": b2r[128:].reshape(128, 1).astype(np.float32),
    }


_NC_CACHE = {}


def _get_program(n_rows, b):
    key = (n_rows, b)
    if key not in _NC_CACHE:
        _NC_CACHE[key] = build_program(n_rows, b)
    return _NC_CACHE[key]


def make_in_maps(inputs):
    input_ = np.asarray(inputs["input_"], np.float32)
    n = input_.shape[0]
    n_rows = n // N_CORES
    consts = make_consts(inputs["W1"], inputs["b1"], inputs["W2"], inputs["b2"])
    bfl = ml_dtypes.bfloat16
    in_maps = []
    for c in range(N_CORES):
        sh = input_[c * n_rows:(c + 1) * n_rows]          # [n_rows, 128]
        xt = np.ascontiguousarray(sh[:, :64].T)           # [64, n_rows]
        # pack subtile pairs: rows 0:64 even subtiles, 64:128 odd subtiles
        xt3 = xt.reshape(64, n_rows // B // 2, 2, B)
        xt2 = np.concatenate([xt3[:, :, 0, :], xt3[:, :, 1, :]],
                             axis=0).reshape(128, n_rows // 2)
        vt = np.ascontiguousarray(sh[:, 64:].T)           # [64, n_rows]
        vr = np.concatenate([vt, vt], axis=0)             # [128, n_rows]
        m = {"xt": np.ascontiguousarray(xt2).astype(bfl),
             "vr": np.ascontiguousarray(vr).astype(bfl)}
        m.update(consts)
        in_maps.append(m)
    return in_maps


def kernel(t, input_, W1, b1, W2, b2):
    input_ = np.asarray(input_, np.float32)
    n = input_.shape[0]
    n_rows = n // N_CORES
    nc = _get_program(n_rows, B)
    in_maps = make_in_maps(
        {"input_": input_, "W1": W1, "b1": b1, "W2": W2, "b2": b2})
    res = run_bass_kernel_spmd(nc, in_maps, list(range(N_CORES)))
    out = np.concatenate(
        [np.asarray(res.results[c]["out"]).astype(np.float32).T
         for c in range(N_CORES)], axis=0)
    return out
